# revision 23
# baseline (speedup 1.0000x reference)
"""Trainium2 Bass kernel for the DiffsolClassifier model (v3).

Network (per image, NCHW fp32):
    z1 = relu(conv2d(x, W1, b1, k=3, s=2, p=1))   # [8,14,14]
    z2 = relu(conv2d(z1, W2, b2, k=3, s=2, p=1))  # [16,7,7]
    t  = flatten(z2) @ Wfc.T + bfc                # [1]
    p  = clip(1 - exp(-(softplus(t) + 1e-3)), 1e-6, 1-1e-6)
       = 1 - k*sigmoid(-t),  k = exp(-1e-3)       (clip is a no-op)

Sharding: pure data parallel, batch 65536 split 8192/core across 8 cores.

Per-core mapping (16 outer tiles x 512 images), fp16 data / fp32 PSUM:
  - conv1: host stages overlapping 84-pixel windows; 14 matmuls/tile with
    one shared stationary W1win [84,112]; PSUM pairs [112,2,512] so each
    bias+relu eviction covers TWO rows (fewer, bigger ACT/DVE ops).
  - conv2: banded tap mats [112,112] x 3; rows 0..6 accumulate 2-3 taps
    in PSUM; bias+relu eviction to z2 [112,7,512] fp16.
  - fc (4x col-tiled): 7 r-matmuls collapse into 2 PE slots of concurrent
    M=32 matmuls on col groups (tile_position=(0,32g)), accumulating 8
    tiles into ONE psum bank (stationary column = tile index).  Per 8
    tiles: one [104,512] eviction, a 4-way SBUF DMA gather, 2 DVE adds,
    one batched sigmoid [8,512], one gpsimd affine, one output DMA.
  - warmup: dummy matmuls on a zeroed tile spin the PE HAM throttle to
    8/8 during the initial DMA wait; tile 0's input DMA is split into
    oi-chunks so real conv1 starts ~2-3us in, already warm.
"""

import numpy as np

B = 65536
NCORES = 8
BS = B // NCORES  # 8192 images per core
TN = 512          # images per outer tile
NT = BS // TN     # 16 outer tiles
GRP = 8           # tiles per fc/epilogue group

KDEC = float(np.exp(np.float32(-0.001)))

# set by test.py for profiling; harness leaves these alone
TRACE = False
LAST_EXEC_NS = None
LAST_PROFILE_JSON = None


def _build_weight_mats(W1, b1, W2, b2, Wfc):
    """Host-side restructuring of the tiny conv/fc weights."""
    W1 = np.asarray(W1, np.float32).reshape(8, 1, 3, 3)
    W2 = np.asarray(W2, np.float32).reshape(16, 8, 3, 3)
    Wfc = np.asarray(Wfc, np.float32).reshape(1, 784)

    # W1win[w, (co,oj)] over an 84-pixel window, w = 28*di + (2*oj-1+dj)
    W1win = np.zeros((84, 112), np.float32)
    for co in range(8):
        for oj in range(14):
            m = co * 14 + oj
            for di in range(3):
                for dj in range(3):
                    j = 2 * oj - 1 + dj
                    if 0 <= j < 28:
                        W1win[28 * di + j, m] = W1[co, 0, di, dj]

    # conv2 tap matrices: W2r[di][(ci,j), (co2,oj2)]
    W2r = np.zeros((3, 112, 112), np.float32)
    for di in range(3):
        for co in range(16):
            for oj in range(7):
                m = co * 7 + oj
                for ci in range(8):
                    for dj in range(3):
                        j = 2 * oj - 1 + dj
                        if 0 <= j < 14:
                            W2r[di, ci * 14 + j, m] = W2[co, ci, di, dj]

    # fc col-tiled stationaries, negated (p = 1 - k*sigmoid(-t) trick).
    # Sfc[r][p, k, c] = -Wfc for column c==k (k = tile index within the
    # 8-tile psum accumulation group); slot1 r=0..3 -> col group r,
    # slot2 r=4..6 -> col group r-4.
    wfc = np.zeros((112, 7), np.float32)
    for co in range(16):
        for i2 in range(7):
            for oj in range(7):
                wfc[co * 7 + oj, i2] = -Wfc[0, co * 49 + i2 * 7 + oj]
    Sfc = np.zeros((7, 112, GRP, 32), np.float32)
    for r in range(7):
        for k in range(GRP):
            Sfc[r, :, k, k] = wfc[:, r]

    b1col = np.repeat(np.asarray(b1, np.float32), 14).reshape(112, 1)
    b2col = np.repeat(np.asarray(b2, np.float32), 7).reshape(112, 1)
    return W1win, W2r, Sfc, b1col, b2col


def _build_nc(nt_tiles):
    import concourse.bacc as bacc
    import concourse.bass as bass
    import concourse.mybir as mybir
    import concourse.tile as tile

    f32 = mybir.dt.float32
    f16 = mybir.dt.float16
    AF = mybir.ActivationFunctionType
    OP = mybir.AluOpType
    bs = nt_tiles * TN
    ngrp = (nt_tiles + GRP - 1) // GRP

    nc = bacc.Bacc(None)
    xw_d = nc.declare_dram_parameter("xw", [nt_tiles, 2, 588, TN], f16,
                                     isOutput=False)
    w1_d = nc.declare_dram_parameter("w1win", [84, 112], f16, isOutput=False)
    w2r0_d = nc.declare_dram_parameter("w2r0", [112, 112], f16, isOutput=False)
    w2r1_d = nc.declare_dram_parameter("w2r1", [112, 112], f16, isOutput=False)
    w2r2_d = nc.declare_dram_parameter("w2r2", [112, 112], f16, isOutput=False)
    sfc_d = nc.declare_dram_parameter("sfc", [112, 7, GRP * 32], f16,
                                      isOutput=False)
    b1_d = nc.declare_dram_parameter("b1col", [112, 1], f32, isOutput=False)
    b2_d = nc.declare_dram_parameter("b2col", [112, 1], f32, isOutput=False)
    bfc_d = nc.declare_dram_parameter("bfcneg", [GRP, 1], f32, isOutput=False)
    y_d = nc.declare_dram_parameter("y", [bs], f32, isOutput=True)

    with tile.TileContext(nc) as tc:
        with (
            tc.tile_pool(name="const", bufs=1) as const,
            tc.tile_pool(name="xt_pool", bufs=4) as xt_pool,
            tc.tile_pool(name="z1_pool", bufs=3) as z1_pool,
            tc.tile_pool(name="z2_pool", bufs=3) as z2_pool,
            tc.tile_pool(name="fcs_pool", bufs=2) as fcs_pool,
            tc.tile_pool(name="c1_psum", bufs=4, space="PSUM") as c1_pool,
            tc.tile_pool(name="c2_psum", bufs=3, space="PSUM") as c2_pool,
            tc.tile_pool(name="fc_psum", bufs=1, space="PSUM") as fc_pool,
        ):
            w1win = const.tile([84, 112], f16, name="w1win")
            w2r = [const.tile([112, 112], f16, tag=f"w2r{i}", name=f"w2r{i}")
                   for i in range(3)]
            sfc = const.tile([112, 7, GRP * 32], f16, tag="sfc", name="sfc")
            b1 = const.tile([112, 1], f32, tag="b1", name="b1")
            b2 = const.tile([112, 1], f32, tag="b2", name="b2")
            bfc = const.tile([GRP, 1], f32, tag="bfc", name="bfc")
            dummy = const.tile([84, 640], f16, tag="dummy", name="dummy")
            # zero the warmup tile on the (idle) vector queue immediately
            nc.vector.memset(dummy[:], 0.0)

            # weight loads issue from the (otherwise idle) GPSIMD queue so
            # neither the SP queue (input tiles) nor the ACT queue (first
            # evictions) is blocked at startup
            for sb, dr in [(w1win, w1_d), (b1, b1_d), (w2r[0], w2r0_d),
                           (w2r[1], w2r1_d), (w2r[2], w2r2_d),
                           (b2, b2_d), (bfc, bfc_d), (sfc, sfc_d)]:
                nc.gpsimd.dma_start(out=sb[:], in_=dr[:])

            # fc psum bank: one [128,512] bank accumulating GRP tiles
            fcps = fc_pool.tile([128, TN], f32, tag="fc", name="fc")

            # ---- PE warmup: spin HAM to 8/8 during the first DMA wait ----
            # dummy zero matmuls, same (128,128) tile config as conv1.  The
            # initial burst covers the queue-preamble -> first-data window
            # (~8-16us); smaller bursts are interleaved into tiles 0/1 so no
            # DMA-wait window exceeds HAM's ~3.4us idle threshold.
            def dummy_mms(n):
                for w in range(n):
                    nc.tensor.matmul(fcps[0:112, :], dummy[:, 0:112],
                                     dummy[:, 128:640], start=True,
                                     stop=True)

            dummy_mms(20)

            # alternate PSUM->SBUF bias+relu evictions across ACT and DVE
            evict_i = [0]

            def evict_relu(dst, src, bias):
                evict_i[0] += 1
                if evict_i[0] % 2:
                    nc.scalar.activation(dst, src, AF.Relu, bias=bias[:, 0:1])
                else:
                    nc.vector.tensor_scalar(dst, src, bias[:, 0:1], 0.0,
                                            OP.add, OP.max)

            def fc_mms(t, z2t, first, last):
                """Col-tiled fc matmuls for tile t into fcps (col = t%GRP)."""
                k = t % GRP
                for r in range(7):
                    g = r if r < 4 else r - 4
                    # the PSUM has_written clear is per written partition
                    # region, so each col group's first/last writer in the
                    # 8-tile accumulation group carries start/stop
                    st = (first and r < 4)
                    sp = (last and r >= 3)
                    nc.tensor.matmul(
                        fcps[32 * g:32 * g + 32, :],
                        sfc[:, r, 32 * k:32 * k + 32],
                        z2t[:, r, :],
                        start=st, stop=sp,
                        tile_position=(0, 32 * g),
                        skip_group_check=True)

            def epilogue(grp_idx):
                """Per-GRP-tiles: reduce 4 col-group partials, sigmoid,
                affine, store GRP*TN outputs."""
                fcsb = fcs_pool.tile([104, TN], f32, tag="fcsb", name="fcsb")
                fcg = fcs_pool.tile([GRP, 4, TN], f32, tag="fcg", name="fcg")
                ut = fcs_pool.tile([GRP, 2, TN], f32, tag="ut", name="ut")
                ysb = fcs_pool.tile([GRP, TN], f32, tag="ysb", name="ysb")
                # single full-width eviction of the fc bank (raw copy)
                nc.scalar.copy(fcsb[:], fcps[0:104, :])
                # gather the 4 col-group partials onto partitions 0..GRP-1
                for g in range(4):
                    nc.gpsimd.dma_start(out=fcg[:, g, :],
                                        in_=fcsb[32 * g:32 * g + GRP, :])
                # tree-reduce on DVE (same-partition ops only)
                nc.vector.tensor_tensor(ut[:], fcg[:, 0:2, :], fcg[:, 2:4, :],
                                        OP.add)
                nc.vector.tensor_tensor(ysb[:], ut[:, 0, :], ut[:, 1, :],
                                        OP.add)
                # sigma(-t) = sigmoid(partialsum + (-bfc))
                nc.scalar.activation(ysb[:], ysb[:], AF.Sigmoid,
                                     bias=bfc[:, 0:1])
                # p = 1 - k*sigma  (GPSIMD: SBUF-only op, engine idle)
                nc.gpsimd.tensor_scalar(ysb[:], ysb[:], -KDEC, 1.0,
                                        OP.mult, OP.add)
                nc.sync.dma_start(out=y_d[bass.ds(grp_idx * GRP * TN,
                                                  GRP * TN)],
                                  in_=ysb[:])

            # each dma_start descriptor streams at ~113 GB/s and descriptors
            # on the SAME queue serialize, so the two halves of each tile
            # ride the two hardware-DGE queues (SP=sync, Activation=scalar;
            # gpsimd DMA is slow software-DGE, only fit for tiny weights).
            # Tiles 0-3 are pre-issued before any compute reaches the
            # queues so the startup is never DMA-starved.
            def dma_half(xt, t, h):
                q = nc.sync if h == 0 else nc.scalar
                q.dma_start(out=xt[:, 7 * h:7 * h + 7, :],
                            in_=xw_d[t, h].rearrange("(p o) n -> p o n",
                                                     p=84))

            xts = {}
            for tt in range(4):
                xts[tt] = xt_pool.tile([84, 14, TN], f16, tag="xt",
                                       name="xt")
                dma_half(xts[tt], tt, 0)
                dma_half(xts[tt], tt, 1)

            z2_hist = []
            for t in range(nt_tiles):
                # prefetch tile t+2's input two iterations ahead so the
                # transfer has ~2 tile-periods of lead over its consumers
                tp2 = t + 2
                if tp2 < nt_tiles and tp2 not in xts:
                    xts[tp2] = xt_pool.tile([84, 14, TN], f16, tag="xt",
                                            name="xt")
                    dma_half(xts[tp2], tp2, 0)
                    dma_half(xts[tp2], tp2, 1)
                xt = xts.pop(t)

                # ---- conv1: one shared stationary, 14 matmuls ----
                z1 = z1_pool.tile([112, 14, TN], f16, tag="z1", name="z1")
                for oi in range(14):
                    p1 = c1_pool.tile([112, TN], f32, tag="p1", name="p1")
                    nc.tensor.matmul(p1[:], w1win[:], xt[:, oi, :],
                                     start=True, stop=True)
                    evict_relu(z1[:, oi, :], p1[:], b1)
                    if t == 0 and oi in (6, 13):
                        dummy_mms(6 if oi == 6 else 4)
                    elif t == 1 and oi == 6:
                        dummy_mms(3)

                # ---- conv2: 20 tap matmuls, per-row eviction ----
                z2 = z2_pool.tile([112, 7, TN], f16, tag="z2", name="z2")
                for r in range(7):
                    dis = [di for di in range(3) if 0 <= 2 * r - 1 + di < 14]
                    p2 = c2_pool.tile([112, TN], f32, tag="p2", name="p2")
                    for k, di in enumerate(dis):
                        nc.tensor.matmul(p2[:], w2r[di][:],
                                         z1[:, 2 * r - 1 + di, :],
                                         start=(k == 0),
                                         stop=(k == len(dis) - 1))
                    evict_relu(z2[:, r, :], p2[:], b2)
                    if t == 0 and r == 3:
                        dummy_mms(4)

                # ---- fc, deferred TWO tiles: its z2 is long evicted and the
                # group-boundary psum copy has a full tile of slack before
                # the next group's start=True matmuls need the bank ----
                z2_hist.append(z2)
                if t >= 2:
                    tp = t - 2
                    fc_mms(tp, z2_hist[tp], first=(tp % GRP == 0),
                           last=(tp % GRP == GRP - 1))
                    if tp % GRP == GRP - 1:
                        epilogue(tp // GRP)

            for tp in (nt_tiles - 2, nt_tiles - 1):
                fc_mms(tp, z2_hist[tp], first=(tp % GRP == 0),
                       last=(tp % GRP == GRP - 1))
            epilogue(nt_tiles // GRP - 1)

    nc.finalize()
    return nc


_NC_CACHE = {}


def _get_nc(nt_tiles):
    if nt_tiles not in _NC_CACHE:
        _NC_CACHE[nt_tiles] = _build_nc(nt_tiles)
    return _NC_CACHE[nt_tiles]


def _stage_x(x):
    """Host-side window staging: xw[core][t, h, p*7+(oi-7h), n] =
    x[core*8192 + t*512 + n, 56*oi - 28 + p], zeros when out of range."""
    x = np.asarray(x, np.float32).reshape(B, 784).astype(np.float16)
    # rows ordered (h, p, oi_local): oi = 7*h + oi_local
    h_idx = np.arange(1176) // 588
    p_idx = (np.arange(1176) % 588) // 7
    oi_idx = 7 * h_idx + (np.arange(1176) % 7)
    px = 56 * oi_idx - 28 + p_idx               # may be negative (oi=0, p<28)
    valid = px >= 0
    xg = np.zeros((B, 1176), np.float16)
    xg[:, valid] = x[:, px[valid]]
    # [B, 1176] -> [NCORES, NT, 1176, TN] -> [NCORES, NT, 2, 588, TN]
    xg = xg.reshape(NCORES, NT, TN, 1176).transpose(0, 1, 3, 2)
    return np.ascontiguousarray(xg).reshape(NCORES, NT, 2, 588, TN)


def _install_trace_hook():
    """Register the axon NTFF profiling hook (test-time only)."""
    import contextlib
    import ctypes
    import sys
    import types

    if "antenv.axon_hooks" in sys.modules:
        return
    try:
        lib = ctypes.CDLL("/opt/axon/libaxon_pjrt.so")
        if not hasattr(lib, "axon_start_nrt_profile"):
            return
        lib.axon_start_nrt_profile.argtypes = [
            ctypes.POINTER(ctypes.c_int64), ctypes.c_size_t]
        lib.axon_start_nrt_profile.restype = ctypes.c_int64
        lib.axon_stop_nrt_profile.argtypes = [ctypes.c_char_p]
        lib.axon_stop_nrt_profile.restype = ctypes.c_int64

        @contextlib.contextmanager
        def _hook(output_dir, device_ids):
            import jax
            jax.devices()
            if device_ids:
                ids = (ctypes.c_int64 * len(device_ids))(*device_ids)
                rc = lib.axon_start_nrt_profile(ids, len(device_ids))
            else:
                rc = lib.axon_start_nrt_profile(None, 0)
            if rc != 0:
                raise RuntimeError(f"axon_start_nrt_profile rc={rc}")
            try:
                yield
            finally:
                rc = lib.axon_stop_nrt_profile(output_dir.encode())
                if rc not in (0, 3):
                    raise RuntimeError(f"axon_stop_nrt_profile rc={rc}")

        mod = types.ModuleType("antenv.axon_hooks")
        mod.get_axon_ntff_profile_hook = lambda: _hook
        mod.set_axon_ntff_profile_hook = lambda h: None
        sys.modules["antenv.axon_hooks"] = mod
        import concourse.bass_utils as bu
        bu.upload_artifacts = lambda tmpdir: tmpdir
    except Exception:
        pass


def kernel(x, W1, b1, W2, b2, Wfc, bfc):
    global LAST_EXEC_NS, LAST_PROFILE_JSON
    from concourse.bass_utils import run_bass_kernel_spmd

    xw = _stage_x(x)
    W1win, W2r, Sfc, b1col, b2col = _build_weight_mats(W1, b1, W2, b2, Wfc)
    bfcneg = np.full((GRP, 1), -np.float32(np.asarray(bfc).reshape(())),
                     np.float32)

    nc = _get_nc(NT)
    shared = {
        "w1win": W1win.astype(np.float16),
        "w2r0": np.ascontiguousarray(W2r[0]).astype(np.float16),
        "w2r1": np.ascontiguousarray(W2r[1]).astype(np.float16),
        "w2r2": np.ascontiguousarray(W2r[2]).astype(np.float16),
        "sfc": np.ascontiguousarray(
            Sfc.reshape(7, 112, GRP * 32).transpose(1, 0, 2)).astype(
            np.float16),
        "b1col": b1col, "b2col": b2col, "bfcneg": bfcneg,
    }
    in_maps = [{"xw": xw[i], **shared} for i in range(NCORES)]
    core_ids = list(range(NCORES))
    res = run_bass_kernel_spmd(nc, in_maps, core_ids)
    y = np.concatenate([res.results[i]["y"] for i in range(NCORES)])

    if TRACE:
        _install_trace_hook()
        try:
            tres = run_bass_kernel_spmd(nc, in_maps, core_ids, trace=True)
            LAST_EXEC_NS = tres.exec_time_ns
            LAST_PROFILE_JSON = tres.profile_json
        except Exception as e:  # profiling must never break the result path
            print("trace failed:", e)

    return y.astype(np.float32)


# revision 25
# speedup vs baseline: 1.0064x; 1.0064x over previous
"""Trainium2 Bass kernel for the DiffsolClassifier model (v3).

Network (per image, NCHW fp32):
    z1 = relu(conv2d(x, W1, b1, k=3, s=2, p=1))   # [8,14,14]
    z2 = relu(conv2d(z1, W2, b2, k=3, s=2, p=1))  # [16,7,7]
    t  = flatten(z2) @ Wfc.T + bfc                # [1]
    p  = clip(1 - exp(-(softplus(t) + 1e-3)), 1e-6, 1-1e-6)
       = 1 - k*sigmoid(-t),  k = exp(-1e-3)       (clip is a no-op)

Sharding: pure data parallel, batch 65536 split 8192/core across 8 cores.

Per-core mapping (16 outer tiles x 512 images), fp16 data / fp32 PSUM:
  - conv1: host stages overlapping 84-pixel windows; 14 matmuls/tile with
    one shared stationary W1win [84,112]; PSUM pairs [112,2,512] so each
    bias+relu eviction covers TWO rows (fewer, bigger ACT/DVE ops).
  - conv2: banded tap mats [112,112] x 3; rows 0..6 accumulate 2-3 taps
    in PSUM; bias+relu eviction to z2 [112,7,512] fp16.
  - fc (4x col-tiled): 7 r-matmuls collapse into 2 PE slots of concurrent
    M=32 matmuls on col groups (tile_position=(0,32g)), accumulating 8
    tiles into ONE psum bank (stationary column = tile index).  Per 8
    tiles: one [104,512] eviction, a 4-way SBUF DMA gather, 2 DVE adds,
    one batched sigmoid [8,512], one gpsimd affine, one output DMA.
  - warmup: dummy matmuls on a zeroed tile spin the PE HAM throttle to
    8/8 during the initial DMA wait; tile 0's input DMA is split into
    oi-chunks so real conv1 starts ~2-3us in, already warm.
"""

import numpy as np

B = 65536
NCORES = 8
BS = B // NCORES  # 8192 images per core
TN = 512          # images per outer tile
NT = BS // TN     # 16 outer tiles
GRP = 8           # tiles per fc/epilogue group

KDEC = float(np.exp(np.float32(-0.001)))

# set by test.py for profiling; harness leaves these alone
TRACE = False
TRACE_REPS = 1
LAST_EXEC_NS = None
LAST_EXEC_NS_ALL = None
LAST_PROFILE_JSON = None


def _build_weight_mats(W1, b1, W2, b2, Wfc):
    """Host-side restructuring of the tiny conv/fc weights."""
    W1 = np.asarray(W1, np.float32).reshape(8, 1, 3, 3)
    W2 = np.asarray(W2, np.float32).reshape(16, 8, 3, 3)
    Wfc = np.asarray(Wfc, np.float32).reshape(1, 784)

    # W1win[w, (co,oj)] over an 84-pixel window, w = 28*di + (2*oj-1+dj)
    W1win = np.zeros((84, 112), np.float32)
    for co in range(8):
        for oj in range(14):
            m = co * 14 + oj
            for di in range(3):
                for dj in range(3):
                    j = 2 * oj - 1 + dj
                    if 0 <= j < 28:
                        W1win[28 * di + j, m] = W1[co, 0, di, dj]

    # conv2 tap matrices: W2r[di][(ci,j), (co2,oj2)]
    W2r = np.zeros((3, 112, 112), np.float32)
    for di in range(3):
        for co in range(16):
            for oj in range(7):
                m = co * 7 + oj
                for ci in range(8):
                    for dj in range(3):
                        j = 2 * oj - 1 + dj
                        if 0 <= j < 14:
                            W2r[di, ci * 14 + j, m] = W2[co, ci, di, dj]

    # fc col-tiled stationaries, negated (p = 1 - k*sigmoid(-t) trick).
    # Sfc[r][p, k, c] = -Wfc for column c==k (k = tile index within the
    # 8-tile psum accumulation group); slot1 r=0..3 -> col group r,
    # slot2 r=4..6 -> col group r-4.
    wfc = np.zeros((112, 7), np.float32)
    for co in range(16):
        for i2 in range(7):
            for oj in range(7):
                wfc[co * 7 + oj, i2] = -Wfc[0, co * 49 + i2 * 7 + oj]
    Sfc = np.zeros((7, 112, GRP, 32), np.float32)
    for r in range(7):
        for k in range(GRP):
            Sfc[r, :, k, k] = wfc[:, r]

    b1col = np.repeat(np.asarray(b1, np.float32), 14).reshape(112, 1)
    b2col = np.repeat(np.asarray(b2, np.float32), 7).reshape(112, 1)
    return W1win, W2r, Sfc, b1col, b2col


def _build_nc(nt_tiles):
    import concourse.bacc as bacc
    import concourse.bass as bass
    import concourse.mybir as mybir
    import concourse.tile as tile

    f32 = mybir.dt.float32
    f16 = mybir.dt.float16
    AF = mybir.ActivationFunctionType
    OP = mybir.AluOpType
    bs = nt_tiles * TN
    ngrp = (nt_tiles + GRP - 1) // GRP

    nc = bacc.Bacc(None)
    xw_d = nc.declare_dram_parameter("xw", [nt_tiles, 2, 588, TN], f16,
                                     isOutput=False)
    w1_d = nc.declare_dram_parameter("w1win", [84, 112], f16, isOutput=False)
    w2r0_d = nc.declare_dram_parameter("w2r0", [112, 112], f16, isOutput=False)
    w2r1_d = nc.declare_dram_parameter("w2r1", [112, 112], f16, isOutput=False)
    w2r2_d = nc.declare_dram_parameter("w2r2", [112, 112], f16, isOutput=False)
    sfc_d = nc.declare_dram_parameter("sfc", [112, 7, GRP * 32], f16,
                                      isOutput=False)
    b1_d = nc.declare_dram_parameter("b1col", [112, 1], f32, isOutput=False)
    b2_d = nc.declare_dram_parameter("b2col", [112, 1], f32, isOutput=False)
    bfc_d = nc.declare_dram_parameter("bfcneg", [GRP, 1], f32, isOutput=False)
    y_d = nc.declare_dram_parameter("y", [bs], f32, isOutput=True)

    with tile.TileContext(nc) as tc:
        with (
            tc.tile_pool(name="const", bufs=1) as const,
            tc.tile_pool(name="xt_pool", bufs=4) as xt_pool,
            tc.tile_pool(name="z1_pool", bufs=3) as z1_pool,
            tc.tile_pool(name="z2_pool", bufs=3) as z2_pool,
            tc.tile_pool(name="fcs_pool", bufs=2) as fcs_pool,
            tc.tile_pool(name="c1_psum", bufs=4, space="PSUM") as c1_pool,
            tc.tile_pool(name="c2_psum", bufs=3, space="PSUM") as c2_pool,
            tc.tile_pool(name="fc_psum", bufs=1, space="PSUM") as fc_pool,
        ):
            w1win = const.tile([84, 112], f16, name="w1win")
            w2r = [const.tile([112, 112], f16, tag=f"w2r{i}", name=f"w2r{i}")
                   for i in range(3)]
            sfc = const.tile([112, 7, GRP * 32], f16, tag="sfc", name="sfc")
            b1 = const.tile([112, 1], f32, tag="b1", name="b1")
            b2 = const.tile([112, 1], f32, tag="b2", name="b2")
            bfc = const.tile([GRP, 1], f32, tag="bfc", name="bfc")
            dummy = const.tile([84, 640], f16, tag="dummy", name="dummy")
            # zero the warmup tile on the (idle) vector queue immediately
            nc.vector.memset(dummy[:], 0.0)

            # weight loads issue from the (otherwise idle) GPSIMD queue so
            # neither the SP queue (input tiles) nor the ACT queue (first
            # evictions) is blocked at startup
            for sb, dr in [(w1win, w1_d), (b1, b1_d), (w2r[0], w2r0_d),
                           (w2r[1], w2r1_d), (w2r[2], w2r2_d),
                           (b2, b2_d), (bfc, bfc_d), (sfc, sfc_d)]:
                nc.gpsimd.dma_start(out=sb[:], in_=dr[:])

            # fc psum bank: one [128,512] bank accumulating GRP tiles
            fcps = fc_pool.tile([128, TN], f32, tag="fc", name="fc")

            # ---- PE warmup: spin HAM to 8/8 during the first DMA wait ----
            # dummy zero matmuls, same (128,128) tile config as conv1.  The
            # initial burst covers the queue-preamble -> first-data window
            # (~8-16us); smaller bursts are interleaved into tiles 0/1 so no
            # DMA-wait window exceeds HAM's ~3.4us idle threshold.
            def dummy_mms(n):
                for w in range(n):
                    nc.tensor.matmul(fcps[0:112, :], dummy[:, 0:112],
                                     dummy[:, 128:640], start=True,
                                     stop=True)

            dummy_mms(20)

            # alternate PSUM->SBUF bias+relu evictions across ACT and DVE
            evict_i = [0]

            def evict_relu(dst, src, bias):
                evict_i[0] += 1
                if evict_i[0] % 2:
                    nc.scalar.activation(dst, src, AF.Relu, bias=bias[:, 0:1])
                else:
                    nc.vector.tensor_scalar(dst, src, bias[:, 0:1], 0.0,
                                            OP.add, OP.max)

            def fc_mms(t, z2t, first, last):
                """Col-tiled fc matmuls for tile t into fcps (col = t%GRP)."""
                k = t % GRP
                for r in range(7):
                    g = r if r < 4 else r - 4
                    # the PSUM has_written clear is per written partition
                    # region, so each col group's first/last writer in the
                    # 8-tile accumulation group carries start/stop
                    st = (first and r < 4)
                    sp = (last and r >= 3)
                    nc.tensor.matmul(
                        fcps[32 * g:32 * g + 32, :],
                        sfc[:, r, 32 * k:32 * k + 32],
                        z2t[:, r, :],
                        start=st, stop=sp,
                        tile_position=(0, 32 * g),
                        skip_group_check=True)

            def epilogue(grp_idx):
                """Per-GRP-tiles: reduce 4 col-group partials, sigmoid,
                affine, store GRP*TN outputs."""
                fcsb = fcs_pool.tile([104, TN], f32, tag="fcsb", name="fcsb")
                fcg = fcs_pool.tile([GRP, 4, TN], f32, tag="fcg", name="fcg")
                ut = fcs_pool.tile([GRP, 2, TN], f32, tag="ut", name="ut")
                ysb = fcs_pool.tile([GRP, TN], f32, tag="ysb", name="ysb")
                # single full-width eviction of the fc bank (raw copy)
                nc.scalar.copy(fcsb[:], fcps[0:104, :])
                # gather the 4 col-group partials onto partitions 0..GRP-1
                for g in range(4):
                    nc.gpsimd.dma_start(out=fcg[:, g, :],
                                        in_=fcsb[32 * g:32 * g + GRP, :])
                # tree-reduce on DVE (same-partition ops only)
                nc.vector.tensor_tensor(ut[:], fcg[:, 0:2, :], fcg[:, 2:4, :],
                                        OP.add)
                nc.vector.tensor_tensor(ysb[:], ut[:, 0, :], ut[:, 1, :],
                                        OP.add)
                # sigma(-t) = sigmoid(partialsum + (-bfc))
                nc.scalar.activation(ysb[:], ysb[:], AF.Sigmoid,
                                     bias=bfc[:, 0:1])
                # p = 1 - k*sigma  (GPSIMD: SBUF-only op, engine idle)
                nc.gpsimd.tensor_scalar(ysb[:], ysb[:], -KDEC, 1.0,
                                        OP.mult, OP.add)
                nc.sync.dma_start(out=y_d[bass.ds(grp_idx * GRP * TN,
                                                  GRP * TN)],
                                  in_=ysb[:])

            # each dma_start descriptor streams at ~113 GB/s and descriptors
            # on the SAME queue serialize, so the two halves of each tile
            # ride the two hardware-DGE queues (SP=sync, Activation=scalar;
            # gpsimd DMA is slow software-DGE, only fit for tiny weights).
            # Tiles 0-3 are pre-issued before any compute reaches the
            # queues so the startup is never DMA-starved.
            def dma_half(xt, t, h):
                q = nc.sync if h == 0 else nc.scalar
                q.dma_start(out=xt[:, 7 * h:7 * h + 7, :],
                            in_=xw_d[t, h].rearrange("(p o) n -> p o n",
                                                     p=84))

            xts = {}
            for tt in range(4):
                xts[tt] = xt_pool.tile([84, 14, TN], f16, tag="xt",
                                       name="xt")
                dma_half(xts[tt], tt, 0)
                dma_half(xts[tt], tt, 1)

            z2_hist = []
            for t in range(nt_tiles):
                # prefetch tile t+2's input two iterations ahead so the
                # transfer has ~2 tile-periods of lead over its consumers
                tp2 = t + 2
                if tp2 < nt_tiles and tp2 not in xts:
                    xts[tp2] = xt_pool.tile([84, 14, TN], f16, tag="xt",
                                            name="xt")
                    dma_half(xts[tp2], tp2, 0)
                    dma_half(xts[tp2], tp2, 1)
                xt = xts.pop(t)

                # ---- conv1: one shared stationary, 14 matmuls ----
                z1 = z1_pool.tile([112, 14, TN], f16, tag="z1", name="z1")
                for oi in range(14):
                    p1 = c1_pool.tile([112, TN], f32, tag="p1", name="p1")
                    nc.tensor.matmul(p1[:], w1win[:], xt[:, oi, :],
                                     start=True, stop=True)
                    evict_relu(z1[:, oi, :], p1[:], b1)
                    if t == 0 and oi in (6, 13):
                        dummy_mms(6 if oi == 6 else 4)
                    elif t == 1 and oi == 6:
                        dummy_mms(3)

                # ---- conv2: 20 tap matmuls, per-row eviction ----
                z2 = z2_pool.tile([112, 7, TN], f16, tag="z2", name="z2")
                for r in range(7):
                    dis = [di for di in range(3) if 0 <= 2 * r - 1 + di < 14]
                    p2 = c2_pool.tile([112, TN], f32, tag="p2", name="p2")
                    for k, di in enumerate(dis):
                        nc.tensor.matmul(p2[:], w2r[di][:],
                                         z1[:, 2 * r - 1 + di, :],
                                         start=(k == 0),
                                         stop=(k == len(dis) - 1))
                    evict_relu(z2[:, r, :], p2[:], b2)
                    if t == 0 and r == 3:
                        dummy_mms(4)

                # ---- fc, deferred TWO tiles: its z2 is long evicted and the
                # group-boundary psum copy has a full tile of slack before
                # the next group's start=True matmuls need the bank ----
                z2_hist.append(z2)
                if t >= 2:
                    tp = t - 2
                    fc_mms(tp, z2_hist[tp], first=(tp % GRP == 0),
                           last=(tp % GRP == GRP - 1))
                    if tp % GRP == GRP - 1:
                        epilogue(tp // GRP)

            for tp in (nt_tiles - 2, nt_tiles - 1):
                fc_mms(tp, z2_hist[tp], first=(tp % GRP == 0),
                       last=(tp % GRP == GRP - 1))
            epilogue(nt_tiles // GRP - 1)

    nc.finalize()
    return nc


_NC_CACHE = {}


def _get_nc(nt_tiles):
    if nt_tiles not in _NC_CACHE:
        _NC_CACHE[nt_tiles] = _build_nc(nt_tiles)
    return _NC_CACHE[nt_tiles]


def _stage_x(x):
    """Host-side window staging: xw[core][t, h, p*7+(oi-7h), n] =
    x[core*8192 + t*512 + n, 56*oi - 28 + p], zeros when out of range."""
    x = np.asarray(x, np.float32).reshape(B, 784).astype(np.float16)
    # rows ordered (h, p, oi_local): oi = 7*h + oi_local
    h_idx = np.arange(1176) // 588
    p_idx = (np.arange(1176) % 588) // 7
    oi_idx = 7 * h_idx + (np.arange(1176) % 7)
    px = 56 * oi_idx - 28 + p_idx               # may be negative (oi=0, p<28)
    valid = px >= 0
    xg = np.zeros((B, 1176), np.float16)
    xg[:, valid] = x[:, px[valid]]
    # [B, 1176] -> [NCORES, NT, 1176, TN] -> [NCORES, NT, 2, 588, TN]
    xg = xg.reshape(NCORES, NT, TN, 1176).transpose(0, 1, 3, 2)
    return np.ascontiguousarray(xg).reshape(NCORES, NT, 2, 588, TN)


def _install_trace_hook():
    """Register the axon NTFF profiling hook (test-time only)."""
    import contextlib
    import ctypes
    import sys
    import types

    if "antenv.axon_hooks" in sys.modules:
        return
    try:
        lib = ctypes.CDLL("/opt/axon/libaxon_pjrt.so")
        if not hasattr(lib, "axon_start_nrt_profile"):
            return
        lib.axon_start_nrt_profile.argtypes = [
            ctypes.POINTER(ctypes.c_int64), ctypes.c_size_t]
        lib.axon_start_nrt_profile.restype = ctypes.c_int64
        lib.axon_stop_nrt_profile.argtypes = [ctypes.c_char_p]
        lib.axon_stop_nrt_profile.restype = ctypes.c_int64

        @contextlib.contextmanager
        def _hook(output_dir, device_ids):
            import jax
            jax.devices()
            if device_ids:
                ids = (ctypes.c_int64 * len(device_ids))(*device_ids)
                rc = lib.axon_start_nrt_profile(ids, len(device_ids))
            else:
                rc = lib.axon_start_nrt_profile(None, 0)
            if rc != 0:
                raise RuntimeError(f"axon_start_nrt_profile rc={rc}")
            try:
                yield
            finally:
                rc = lib.axon_stop_nrt_profile(output_dir.encode())
                if rc not in (0, 3):
                    raise RuntimeError(f"axon_stop_nrt_profile rc={rc}")

        mod = types.ModuleType("antenv.axon_hooks")
        mod.get_axon_ntff_profile_hook = lambda: _hook
        mod.set_axon_ntff_profile_hook = lambda h: None
        sys.modules["antenv.axon_hooks"] = mod
        import concourse.bass_utils as bu
        bu.upload_artifacts = lambda tmpdir: tmpdir
    except Exception:
        pass


def kernel(x, W1, b1, W2, b2, Wfc, bfc):
    global LAST_EXEC_NS, LAST_PROFILE_JSON
    from concourse.bass_utils import run_bass_kernel_spmd

    xw = _stage_x(x)
    W1win, W2r, Sfc, b1col, b2col = _build_weight_mats(W1, b1, W2, b2, Wfc)
    bfcneg = np.full((GRP, 1), -np.float32(np.asarray(bfc).reshape(())),
                     np.float32)

    nc = _get_nc(NT)
    shared = {
        "w1win": W1win.astype(np.float16),
        "w2r0": np.ascontiguousarray(W2r[0]).astype(np.float16),
        "w2r1": np.ascontiguousarray(W2r[1]).astype(np.float16),
        "w2r2": np.ascontiguousarray(W2r[2]).astype(np.float16),
        "sfc": np.ascontiguousarray(
            Sfc.reshape(7, 112, GRP * 32).transpose(1, 0, 2)).astype(
            np.float16),
        "b1col": b1col, "b2col": b2col, "bfcneg": bfcneg,
    }
    in_maps = [{"xw": xw[i], **shared} for i in range(NCORES)]
    core_ids = list(range(NCORES))
    res = run_bass_kernel_spmd(nc, in_maps, core_ids)
    y = np.concatenate([res.results[i]["y"] for i in range(NCORES)])

    if TRACE:
        global LAST_EXEC_NS_ALL
        _install_trace_hook()
        try:
            samples = []
            for _ in range(max(1, TRACE_REPS)):
                tres = run_bass_kernel_spmd(nc, in_maps, core_ids,
                                            trace=True)
                samples.append(tres.exec_time_ns)
            LAST_EXEC_NS_ALL = samples
            LAST_EXEC_NS = min(samples)
            LAST_PROFILE_JSON = tres.profile_json
        except Exception as e:  # profiling must never break the result path
            print("trace failed:", e)

    return y.astype(np.float32)


# revision 28
# speedup vs baseline: 1.0334x; 1.0268x over previous
"""Trainium2 Bass kernel for the DiffsolClassifier model (v3).

Network (per image, NCHW fp32):
    z1 = relu(conv2d(x, W1, b1, k=3, s=2, p=1))   # [8,14,14]
    z2 = relu(conv2d(z1, W2, b2, k=3, s=2, p=1))  # [16,7,7]
    t  = flatten(z2) @ Wfc.T + bfc                # [1]
    p  = clip(1 - exp(-(softplus(t) + 1e-3)), 1e-6, 1-1e-6)
       = 1 - k*sigmoid(-t),  k = exp(-1e-3)       (clip is a no-op)

Sharding: pure data parallel, batch 65536 split 8192/core across 8 cores.

Per-core mapping (16 outer tiles x 512 images), fp16 data / fp32 PSUM:
  - conv1: host stages overlapping 84-pixel windows; 14 matmuls/tile with
    one shared stationary W1win [84,112]; PSUM pairs [112,2,512] so each
    bias+relu eviction covers TWO rows (fewer, bigger ACT/DVE ops).
  - conv2: banded tap mats [112,112] x 3; rows 0..6 accumulate 2-3 taps
    in PSUM; bias+relu eviction to z2 [112,7,512] fp16.
  - fc (4x col-tiled): 7 r-matmuls collapse into 2 PE slots of concurrent
    M=32 matmuls on col groups (tile_position=(0,32g)), accumulating 8
    tiles into ONE psum bank (stationary column = tile index).  Per 8
    tiles: one [104,512] eviction, a 4-way SBUF DMA gather, 2 DVE adds,
    one batched sigmoid [8,512], one gpsimd affine, one output DMA.
  - warmup: dummy matmuls on a zeroed tile spin the PE HAM throttle to
    8/8 during the initial DMA wait; tile 0's input DMA is split into
    oi-chunks so real conv1 starts ~2-3us in, already warm.
"""

import numpy as np

B = 65536
NCORES = 8
BS = B // NCORES  # 8192 images per core
TN = 512          # images per outer tile
NT = BS // TN     # 16 outer tiles
GRP = 8           # tiles per fc/epilogue group

KDEC = float(np.exp(np.float32(-0.001)))

# set by test.py for profiling; harness leaves these alone
TRACE = False
TRACE_REPS = 1
LAST_EXEC_NS = None
LAST_EXEC_NS_ALL = None
LAST_PROFILE_JSON = None


def _build_weight_mats(W1, b1, W2, b2, Wfc):
    """Host-side restructuring of the tiny conv/fc weights."""
    W1 = np.asarray(W1, np.float32).reshape(8, 1, 3, 3)
    W2 = np.asarray(W2, np.float32).reshape(16, 8, 3, 3)
    Wfc = np.asarray(Wfc, np.float32).reshape(1, 784)

    # W1win[w, (co,oj)] over an 84-pixel window, w = 28*di + (2*oj-1+dj)
    W1win = np.zeros((84, 112), np.float32)
    for co in range(8):
        for oj in range(14):
            m = co * 14 + oj
            for di in range(3):
                for dj in range(3):
                    j = 2 * oj - 1 + dj
                    if 0 <= j < 28:
                        W1win[28 * di + j, m] = W1[co, 0, di, dj]

    # conv2 tap matrices: W2r[di][(ci,j), (co2,oj2)]
    W2r = np.zeros((3, 112, 112), np.float32)
    for di in range(3):
        for co in range(16):
            for oj in range(7):
                m = co * 7 + oj
                for ci in range(8):
                    for dj in range(3):
                        j = 2 * oj - 1 + dj
                        if 0 <= j < 14:
                            W2r[di, ci * 14 + j, m] = W2[co, ci, di, dj]

    # fc col-tiled stationaries, negated (p = 1 - k*sigmoid(-t) trick).
    # Sfc[r][p, k, c] = -Wfc for column c==k (k = tile index within the
    # 8-tile psum accumulation group); slot1 r=0..3 -> col group r,
    # slot2 r=4..6 -> col group r-4.
    wfc = np.zeros((112, 7), np.float32)
    for co in range(16):
        for i2 in range(7):
            for oj in range(7):
                wfc[co * 7 + oj, i2] = -Wfc[0, co * 49 + i2 * 7 + oj]
    Sfc = np.zeros((7, 112, GRP, 32), np.float32)
    for r in range(7):
        for k in range(GRP):
            Sfc[r, :, k, k] = wfc[:, r]

    b1col = np.repeat(np.asarray(b1, np.float32), 14).reshape(112, 1)
    b2col = np.repeat(np.asarray(b2, np.float32), 7).reshape(112, 1)
    return W1win, W2r, Sfc, b1col, b2col


def _build_nc(nt_tiles):
    import concourse.bacc as bacc
    import concourse.bass as bass
    import concourse.mybir as mybir
    import concourse.tile as tile

    f32 = mybir.dt.float32
    f16 = mybir.dt.float16
    AF = mybir.ActivationFunctionType
    OP = mybir.AluOpType
    bs = nt_tiles * TN
    ngrp = (nt_tiles + GRP - 1) // GRP

    nc = bacc.Bacc(None)
    xw_d = nc.declare_dram_parameter("xw", [nt_tiles, 2, 588, TN], f16,
                                     isOutput=False)
    w1_d = nc.declare_dram_parameter("w1win", [84, 112], f16, isOutput=False)
    w2r0_d = nc.declare_dram_parameter("w2r0", [112, 112], f16, isOutput=False)
    w2r1_d = nc.declare_dram_parameter("w2r1", [112, 112], f16, isOutput=False)
    w2r2_d = nc.declare_dram_parameter("w2r2", [112, 112], f16, isOutput=False)
    sfc_d = nc.declare_dram_parameter("sfc", [112, 7, GRP * 32], f16,
                                      isOutput=False)
    b1_d = nc.declare_dram_parameter("b1col", [112, 1], f32, isOutput=False)
    b2_d = nc.declare_dram_parameter("b2col", [112, 1], f32, isOutput=False)
    bfc_d = nc.declare_dram_parameter("bfcneg", [GRP, 1], f32, isOutput=False)
    y_d = nc.declare_dram_parameter("y", [bs], f32, isOutput=True)

    with tile.TileContext(nc) as tc:
        with (
            tc.tile_pool(name="const", bufs=1) as const,
            tc.tile_pool(name="xt_pool", bufs=4) as xt_pool,
            tc.tile_pool(name="z1_pool", bufs=3) as z1_pool,
            tc.tile_pool(name="z2_pool", bufs=3) as z2_pool,
            tc.tile_pool(name="fcs_pool", bufs=2) as fcs_pool,
            tc.tile_pool(name="c1_psum", bufs=4, space="PSUM") as c1_pool,
            tc.tile_pool(name="c2_psum", bufs=3, space="PSUM") as c2_pool,
            tc.tile_pool(name="fc_psum", bufs=1, space="PSUM") as fc_pool,
        ):
            w1win = const.tile([84, 112], f16, name="w1win")
            w2r = [const.tile([112, 112], f16, tag=f"w2r{i}", name=f"w2r{i}")
                   for i in range(3)]
            sfc = const.tile([112, 7, GRP * 32], f16, tag="sfc", name="sfc")
            b1 = const.tile([112, 1], f32, tag="b1", name="b1")
            b2 = const.tile([112, 1], f32, tag="b2", name="b2")
            bfc = const.tile([GRP, 1], f32, tag="bfc", name="bfc")
            dummy = const.tile([84, 640], f16, tag="dummy", name="dummy")
            # zero the warmup tile on the (idle) vector queue immediately
            nc.vector.memset(dummy[:], 0.0)

            # weight loads issue from the (otherwise idle) GPSIMD queue so
            # neither the SP queue (input tiles) nor the ACT queue (first
            # evictions) is blocked at startup
            for sb, dr in [(w1win, w1_d), (b1, b1_d), (w2r[0], w2r0_d),
                           (w2r[1], w2r1_d), (w2r[2], w2r2_d),
                           (b2, b2_d), (bfc, bfc_d), (sfc, sfc_d)]:
                nc.gpsimd.dma_start(out=sb[:], in_=dr[:])

            # fc psum bank: one [128,512] bank accumulating GRP tiles
            fcps = fc_pool.tile([128, TN], f32, tag="fc", name="fc")

            # ---- PE warmup: spin HAM to 8/8 during the first DMA wait ----
            # dummy zero matmuls, same (128,128) tile config as conv1.  The
            # initial burst covers the queue-preamble -> first-data window
            # (~8-16us); smaller bursts are interleaved into tiles 0/1 so no
            # DMA-wait window exceeds HAM's ~3.4us idle threshold.
            def dummy_mms(n):
                for w in range(n):
                    nc.tensor.matmul(fcps[0:112, :], dummy[:, 0:112],
                                     dummy[:, 128:640], start=True,
                                     stop=True)

            dummy_mms(12)

            # alternate PSUM->SBUF bias+relu evictions across ACT and DVE
            evict_i = [0]

            def evict_relu(dst, src, bias):
                evict_i[0] += 1
                if evict_i[0] % 2:
                    nc.scalar.activation(dst, src, AF.Relu, bias=bias[:, 0:1])
                else:
                    nc.vector.tensor_scalar(dst, src, bias[:, 0:1], 0.0,
                                            OP.add, OP.max)

            def fc_mms(t, z2t, first, last):
                """Col-tiled fc matmuls for tile t into fcps (col = t%GRP)."""
                k = t % GRP
                for r in range(7):
                    g = r if r < 4 else r - 4
                    # the PSUM has_written clear is per written partition
                    # region, so each col group's first/last writer in the
                    # 8-tile accumulation group carries start/stop
                    st = (first and r < 4)
                    sp = (last and r >= 3)
                    nc.tensor.matmul(
                        fcps[32 * g:32 * g + 32, :],
                        sfc[:, r, 32 * k:32 * k + 32],
                        z2t[:, r, :],
                        start=st, stop=sp,
                        tile_position=(0, 32 * g),
                        skip_group_check=True)

            def epilogue(grp_idx):
                """Per-GRP-tiles: reduce 4 col-group partials, sigmoid,
                affine, store GRP*TN outputs."""
                fcsb = fcs_pool.tile([104, TN], f32, tag="fcsb", name="fcsb")
                fcg = fcs_pool.tile([GRP, 4, TN], f32, tag="fcg", name="fcg")
                ut = fcs_pool.tile([GRP, 2, TN], f32, tag="ut", name="ut")
                ysb = fcs_pool.tile([GRP, TN], f32, tag="ysb", name="ysb")
                # single full-width eviction of the fc bank (raw copy)
                nc.scalar.copy(fcsb[:], fcps[0:104, :])
                # gather the 4 col-group partials onto partitions 0..GRP-1
                for g in range(4):
                    nc.gpsimd.dma_start(out=fcg[:, g, :],
                                        in_=fcsb[32 * g:32 * g + GRP, :])
                # tree-reduce on DVE (same-partition ops only)
                nc.vector.tensor_tensor(ut[:], fcg[:, 0:2, :], fcg[:, 2:4, :],
                                        OP.add)
                nc.vector.tensor_tensor(ysb[:], ut[:, 0, :], ut[:, 1, :],
                                        OP.add)
                # sigma(-t) = sigmoid(partialsum + (-bfc))
                nc.scalar.activation(ysb[:], ysb[:], AF.Sigmoid,
                                     bias=bfc[:, 0:1])
                # p = 1 - k*sigma  (GPSIMD: SBUF-only op, engine idle)
                nc.gpsimd.tensor_scalar(ysb[:], ysb[:], -KDEC, 1.0,
                                        OP.mult, OP.add)
                nc.sync.dma_start(out=y_d[bass.ds(grp_idx * GRP * TN,
                                                  GRP * TN)],
                                  in_=ysb[:])

            # each dma_start descriptor streams at ~113 GB/s and descriptors
            # on the SAME queue serialize, so the two halves of each tile
            # ride the two hardware-DGE queues (SP=sync, Activation=scalar;
            # gpsimd DMA is slow software-DGE, only fit for tiny weights).
            # Tiles 0-3 are pre-issued before any compute reaches the
            # queues so the startup is never DMA-starved.
            def dma_half(xt, t, h):
                q = nc.sync if h == 0 else nc.scalar
                q.dma_start(out=xt[:, 7 * h:7 * h + 7, :],
                            in_=xw_d[t, h].rearrange("(p o) n -> p o n",
                                                     p=84))

            xts = {}
            for tt in range(4):
                xts[tt] = xt_pool.tile([84, 14, TN], f16, tag="xt",
                                       name="xt")
                dma_half(xts[tt], tt, 0)
                dma_half(xts[tt], tt, 1)

            z2_hist = []
            for t in range(nt_tiles):
                # prefetch tile t+1's input one iteration ahead so the
                # transfer has ~1 tile-period of lead over its consumers
                tp1 = t + 1
                if tp1 < nt_tiles and tp1 not in xts:
                    xts[tp1] = xt_pool.tile([84, 14, TN], f16, tag="xt",
                                            name="xt")
                    dma_half(xts[tp1], tp1, 0)
                    dma_half(xts[tp1], tp1, 1)
                xt = xts.pop(t)

                # ---- conv1: one shared stationary, 14 matmuls ----
                z1 = z1_pool.tile([112, 14, TN], f16, tag="z1", name="z1")
                for oi in range(14):
                    p1 = c1_pool.tile([112, TN], f32, tag="p1", name="p1")
                    nc.tensor.matmul(p1[:], w1win[:], xt[:, oi, :],
                                     start=True, stop=True)
                    evict_relu(z1[:, oi, :], p1[:], b1)
                    if t == 0 and oi in (6, 13):
                        dummy_mms(4)
                    elif t == 1 and oi == 6:
                        dummy_mms(3)

                # ---- conv2: 20 tap matmuls, per-row eviction ----
                z2 = z2_pool.tile([112, 7, TN], f16, tag="z2", name="z2")
                for r in range(7):
                    dis = [di for di in range(3) if 0 <= 2 * r - 1 + di < 14]
                    p2 = c2_pool.tile([112, TN], f32, tag="p2", name="p2")
                    for k, di in enumerate(dis):
                        nc.tensor.matmul(p2[:], w2r[di][:],
                                         z1[:, 2 * r - 1 + di, :],
                                         start=(k == 0),
                                         stop=(k == len(dis) - 1))
                    evict_relu(z2[:, r, :], p2[:], b2)
                    if t == 0 and r == 3:
                        dummy_mms(4)

                # ---- fc, deferred TWO tiles: its z2 is long evicted and the
                # group-boundary psum copy has a full tile of slack before
                # the next group's start=True matmuls need the bank ----
                z2_hist.append(z2)
                if t >= 2:
                    tp = t - 2
                    fc_mms(tp, z2_hist[tp], first=(tp % GRP == 0),
                           last=(tp % GRP == GRP - 1))
                    if tp % GRP == GRP - 1:
                        epilogue(tp // GRP)

            for tp in (nt_tiles - 2, nt_tiles - 1):
                fc_mms(tp, z2_hist[tp], first=(tp % GRP == 0),
                       last=(tp % GRP == GRP - 1))
            epilogue(nt_tiles // GRP - 1)

    nc.finalize()
    return nc


_NC_CACHE = {}


def _get_nc(nt_tiles):
    if nt_tiles not in _NC_CACHE:
        _NC_CACHE[nt_tiles] = _build_nc(nt_tiles)
    return _NC_CACHE[nt_tiles]


def _stage_x(x):
    """Host-side window staging: xw[core][t, h, p*7+(oi-7h), n] =
    x[core*8192 + t*512 + n, 56*oi - 28 + p], zeros when out of range."""
    x = np.asarray(x, np.float32).reshape(B, 784).astype(np.float16)
    # rows ordered (h, p, oi_local): oi = 7*h + oi_local
    h_idx = np.arange(1176) // 588
    p_idx = (np.arange(1176) % 588) // 7
    oi_idx = 7 * h_idx + (np.arange(1176) % 7)
    px = 56 * oi_idx - 28 + p_idx               # may be negative (oi=0, p<28)
    valid = px >= 0
    xg = np.zeros((B, 1176), np.float16)
    xg[:, valid] = x[:, px[valid]]
    # [B, 1176] -> [NCORES, NT, 1176, TN] -> [NCORES, NT, 2, 588, TN]
    xg = xg.reshape(NCORES, NT, TN, 1176).transpose(0, 1, 3, 2)
    return np.ascontiguousarray(xg).reshape(NCORES, NT, 2, 588, TN)


def _install_trace_hook():
    """Register the axon NTFF profiling hook (test-time only)."""
    import contextlib
    import ctypes
    import sys
    import types

    if "antenv.axon_hooks" in sys.modules:
        return
    try:
        lib = ctypes.CDLL("/opt/axon/libaxon_pjrt.so")
        if not hasattr(lib, "axon_start_nrt_profile"):
            return
        lib.axon_start_nrt_profile.argtypes = [
            ctypes.POINTER(ctypes.c_int64), ctypes.c_size_t]
        lib.axon_start_nrt_profile.restype = ctypes.c_int64
        lib.axon_stop_nrt_profile.argtypes = [ctypes.c_char_p]
        lib.axon_stop_nrt_profile.restype = ctypes.c_int64

        @contextlib.contextmanager
        def _hook(output_dir, device_ids):
            import jax
            jax.devices()
            if device_ids:
                ids = (ctypes.c_int64 * len(device_ids))(*device_ids)
                rc = lib.axon_start_nrt_profile(ids, len(device_ids))
            else:
                rc = lib.axon_start_nrt_profile(None, 0)
            if rc != 0:
                raise RuntimeError(f"axon_start_nrt_profile rc={rc}")
            try:
                yield
            finally:
                rc = lib.axon_stop_nrt_profile(output_dir.encode())
                if rc not in (0, 3):
                    raise RuntimeError(f"axon_stop_nrt_profile rc={rc}")

        mod = types.ModuleType("antenv.axon_hooks")
        mod.get_axon_ntff_profile_hook = lambda: _hook
        mod.set_axon_ntff_profile_hook = lambda h: None
        sys.modules["antenv.axon_hooks"] = mod
        import concourse.bass_utils as bu
        bu.upload_artifacts = lambda tmpdir: tmpdir
    except Exception:
        pass


def kernel(x, W1, b1, W2, b2, Wfc, bfc):
    global LAST_EXEC_NS, LAST_PROFILE_JSON
    from concourse.bass_utils import run_bass_kernel_spmd

    xw = _stage_x(x)
    W1win, W2r, Sfc, b1col, b2col = _build_weight_mats(W1, b1, W2, b2, Wfc)
    bfcneg = np.full((GRP, 1), -np.float32(np.asarray(bfc).reshape(())),
                     np.float32)

    nc = _get_nc(NT)
    shared = {
        "w1win": W1win.astype(np.float16),
        "w2r0": np.ascontiguousarray(W2r[0]).astype(np.float16),
        "w2r1": np.ascontiguousarray(W2r[1]).astype(np.float16),
        "w2r2": np.ascontiguousarray(W2r[2]).astype(np.float16),
        "sfc": np.ascontiguousarray(
            Sfc.reshape(7, 112, GRP * 32).transpose(1, 0, 2)).astype(
            np.float16),
        "b1col": b1col, "b2col": b2col, "bfcneg": bfcneg,
    }
    in_maps = [{"xw": xw[i], **shared} for i in range(NCORES)]
    core_ids = list(range(NCORES))
    res = run_bass_kernel_spmd(nc, in_maps, core_ids)
    y = np.concatenate([res.results[i]["y"] for i in range(NCORES)])

    if TRACE:
        global LAST_EXEC_NS_ALL
        _install_trace_hook()
        try:
            samples = []
            for _ in range(max(1, TRACE_REPS)):
                tres = run_bass_kernel_spmd(nc, in_maps, core_ids,
                                            trace=True)
                samples.append(tres.exec_time_ns)
            LAST_EXEC_NS_ALL = samples
            LAST_EXEC_NS = min(samples)
            LAST_PROFILE_JSON = tres.profile_json
        except Exception as e:  # profiling must never break the result path
            print("trace failed:", e)

    return y.astype(np.float32)
